# revision 8
# baseline (speedup 1.0000x reference)
"""Trainium2 Bass kernel for nn_CONTRASTLoss: squared Pearson-correlation loss
over two 16,777,216-element f32 vectors.

Strategy (data-parallel over 8 NeuronCores):
  - Each core takes a contiguous 2,097,152-element shard of d1 and d2,
    viewed as [128 partitions x 16384 free], streamed in chunks (1 MiB DMAs,
    tapered smaller at the end to shorten the pipeline tail).
  - Per chunk, fused streaming ops compute per-partition partial sums of the
    five sufficient statistics around the known center 0.5 (avoids f32
    catastrophic cancellation in the covariance):
      VectorE : e1 = d1-0.5 (accum -> S1), e2 = d2-0.5 (accum -> S2),
                p = (d1-0.5)*e2 (accum -> P)
      ScalarE : Square(d1-0.5) via the activation bias port (accum -> Q1/Q2),
    so the two compute engines run fully decoupled; both are hidden under
    the HBM-bound DMA stream (~47us/core at ~358 GB/s).
  - Hand-scheduled raw Bass (no Tile framework): static 3-deep double
    buffering with per-buffer-slot DMA semaphores and engine flow-control
    sems. The tail is minimal: one merged [128 x 5*NCH] partials store and
    a GpSimd store-completion wait (the runtime zeroes semaphores at each
    execution start, so no in-kernel reset is needed).
  - Partials (~28 KB/core) are combined on host in float64.
"""
import sys

if '/opt/trn_rl_repo' not in sys.path:
    sys.path.insert(0, '/opt/trn_rl_repo')

import numpy as np

N = 16777216
NCORES = 8
SHARD = N // NCORES      # 2097152
P = 128
FTOT = SHARD // P        # 16384
CHUNKS = [2048] * 6 + [1024, 1024, 1024, 512, 512]
assert sum(CHUNKS) == FTOT
NCH = len(CHUNKS)
MAXF = max(CHUNKS)
TBUFS = 3

_cached_nc = None


def _build():
    import concourse.bacc as bacc
    import concourse.mybir as mybir

    f32 = mybir.dt.float32
    nc = bacc.Bacc('TRN2', target_bir_lowering=False, debug=False)

    cap = nc.alloc_sbuf_tensor("const_neghalf", [P, 1], f32)
    t1b = [nc.alloc_sbuf_tensor(f"t1b{i}", [P, MAXF], f32) for i in range(TBUFS)]
    t2b = [nc.alloc_sbuf_tensor(f"t2b{i}", [P, MAXF], f32) for i in range(TBUFS)]
    e1b = nc.alloc_sbuf_tensor("e1b", [P, MAXF], f32)
    e2b = nc.alloc_sbuf_tensor("e2b", [P, MAXF], f32)
    gpb = nc.alloc_sbuf_tensor("gpb", [P, MAXF], f32)
    sqb = nc.alloc_sbuf_tensor("sqb", [P, MAXF], f32)
    stats_t = nc.alloc_sbuf_tensor("stats_t", [P, 5 * NCH], f32)
    nc.const_aps.aps[(f32, -0.5)] = cap.ap()

    d1 = nc.declare_dram_parameter("d1", [P, FTOT], f32, isOutput=False)
    d2 = nc.declare_dram_parameter("d2", [P, FTOT], f32, isOutput=False)
    out = nc.declare_dram_parameter("stats", [5 * P * NCH], f32, isOutput=True)

    s1sem = [nc.alloc_semaphore(f"s1sem{i}") for i in range(TBUFS)]
    s2sem = [nc.alloc_semaphore(f"s2sem{i}") for i in range(TBUFS)]
    v_sem = nc.alloc_semaphore("v_sem")
    a_sem = nc.alloc_semaphore("a_sem")
    c_sem = nc.alloc_semaphore("c_sem")
    st_sem = nc.alloc_semaphore("st_sem")

    stv = stats_t.ap()
    s1 = stv[:, 0 * NCH:1 * NCH]
    s2 = stv[:, 1 * NCH:2 * NCH]
    pp = stv[:, 2 * NCH:3 * NCH]
    q1 = stv[:, 3 * NCH:4 * NCH]
    q2 = stv[:, 4 * NCH:5 * NCH]

    # --- init: the -0.5 bias constant for ScalarE Square ---
    nc.gpsimd.memset(cap.ap(), -0.5).then_inc(c_sem, 1)

    offs = np.cumsum([0] + CHUNKS)

    # --- SP: all load DMAs, flow-controlled on buffer-slot consumers ---
    for c, fch in enumerate(CHUNKS):
        if c >= TBUFS:
            nc.sync.wait_ge(v_sem, 3 * (c - TBUFS + 1))
            nc.sync.wait_ge(a_sem, 2 * (c - TBUFS + 1))
        lo = int(offs[c])
        t1 = t1b[c % TBUFS].ap()[:, :fch]
        t2 = t2b[c % TBUFS].ap()[:, :fch]
        nc.sync.dma_start(out=t1, in_=d1[:, lo:lo + fch]).then_inc(s1sem[c % TBUFS], 16)
        nc.sync.dma_start(out=t2, in_=d2[:, lo:lo + fch]).then_inc(s2sem[c % TBUFS], 16)

    # --- VectorE: ts(e1)+accum S1, ts(e2)+accum S2, stt +accum P ---
    for c, fch in enumerate(CHUNKS):
        t1 = t1b[c % TBUFS].ap()[:, :fch]
        t2 = t2b[c % TBUFS].ap()[:, :fch]
        e1 = e1b.ap()[:, :fch]
        e2 = e2b.ap()[:, :fch]
        gp = gpb.ap()[:, :fch]
        nc.vector.wait_ge(s1sem[c % TBUFS], 16 * (c // TBUFS + 1))
        nc.vector.tensor_scalar(
            out=e1, in0=t1, scalar1=0.5, scalar2=None,
            op0=mybir.AluOpType.subtract, op1=mybir.AluOpType.add,
            accum_out=s1[:, c:c + 1]).then_inc(v_sem, 1)
        nc.vector.wait_ge(s2sem[c % TBUFS], 16 * (c // TBUFS + 1))
        nc.vector.tensor_scalar(
            out=e2, in0=t2, scalar1=0.5, scalar2=None,
            op0=mybir.AluOpType.subtract, op1=mybir.AluOpType.add,
            accum_out=s2[:, c:c + 1]).then_inc(v_sem, 1)
        nc.vector.scalar_tensor_tensor(
            out=gp, in0=t1, scalar=0.5, in1=e2,
            op0=mybir.AluOpType.subtract, op1=mybir.AluOpType.mult,
            accum_out=pp[:, c:c + 1]).then_inc(v_sem, 1)

    # --- ScalarE: Square(t - 0.5) with accum -> Q1/Q2 ---
    nc.scalar.wait_ge(c_sem, 1)
    for c, fch in enumerate(CHUNKS):
        t1 = t1b[c % TBUFS].ap()[:, :fch]
        t2 = t2b[c % TBUFS].ap()[:, :fch]
        sq = sqb.ap()[:, :fch]
        nc.scalar.wait_ge(s1sem[c % TBUFS], 16 * (c // TBUFS + 1))
        nc.scalar.activation(
            out=sq, in_=t1, func=mybir.ActivationFunctionType.Square,
            bias=-0.5, scale=1.0,
            accum_out=q1[:, c:c + 1]).then_inc(a_sem, 1)
        nc.scalar.wait_ge(s2sem[c % TBUFS], 16 * (c // TBUFS + 1))
        nc.scalar.activation(
            out=sq, in_=t2, func=mybir.ActivationFunctionType.Square,
            bias=-0.5, scale=1.0,
            accum_out=q2[:, c:c + 1]).then_inc(a_sem, 1)

    # --- SP: single merged store once both producer engines finish ---
    ov = out[0:P * 5 * NCH].rearrange("(p c) -> p c", p=P)
    nc.sync.wait_ge(v_sem, 3 * NCH)
    nc.sync.wait_ge(a_sem, 2 * NCH)
    nc.sync.dma_start(out=ov, in_=stv).then_inc(st_sem, 16)

    # GpSimd proves store completion (keeps the kernel alive until the
    # output has landed in DRAM). No in-kernel semaphore reset: the runtime
    # zeroes semaphores at each execution start (verified by alternating
    # different-input executions of a no-clear build on shared devices).
    nc.gpsimd.wait_ge(st_sem, 16)

    nc.finalize()
    return nc


def _run_device(a1, a2, trace=False, tmpdir=None):
    from concourse.bass_utils import run_bass_kernel_spmd

    sh1 = a1.reshape(NCORES, P, FTOT)
    sh2 = a2.reshape(NCORES, P, FTOT)
    in_maps = [{"d1": sh1[c], "d2": sh2[c]} for c in range(NCORES)]
    global _cached_nc
    if _cached_nc is None:
        _cached_nc = _build()
    res = run_bass_kernel_spmd(
        _cached_nc, in_maps, list(range(NCORES)), trace=trace, tmpdir=tmpdir)
    stats = np.stack([res.results[c]["stats"] for c in range(NCORES)])
    return stats, res


def _combine(stats):
    # stats: [NCORES, 5*P*NCH] f32 partials around center 0.5,
    # per-core layout [P, 5, NCH] with stat order [S1, S2, P, Q1, Q2]
    t = stats.astype(np.float64).reshape(NCORES, P, 5, NCH)
    S1 = t[:, :, 0, :].sum()
    S2 = t[:, :, 1, :].sum()
    Pc = t[:, :, 2, :].sum()
    Q1 = t[:, :, 3, :].sum()
    Q2 = t[:, :, 4, :].sum()
    n = float(N)
    mean1c = S1 / n
    mean2c = S2 / n
    a1 = mean1c + 0.001
    a2 = mean2c + 0.001
    var1 = (Q1 - S1 * S1 / n) / (n - 1)
    var2 = (Q2 - S2 * S2 / n) / (n - 1)
    std1 = np.sqrt(var1)
    std2 = np.sqrt(var2)
    cov = (Pc - a2 * S1 - a1 * S2 + n * a1 * a2) / (n - 1)
    cor = cov / (std1 * std2 + 0.001)
    loss = 0.5 * (cor + 0.001) ** 2
    return np.array([loss], dtype=np.float32)


def kernel(distribution1, distribution2):
    a1 = np.ascontiguousarray(np.asarray(distribution1, dtype=np.float32))
    a2 = np.ascontiguousarray(np.asarray(distribution2, dtype=np.float32))
    stats, _ = _run_device(a1, a2)
    return _combine(stats)
